# revision 13
# baseline (speedup 1.0000x reference)
"""Trainium2 Bass kernel for nn_DependencyEncoder (stack TreeLSTM).

Self-contained: takes FULL inputs as in reference.setup_inputs(), shards the
batch across 8 NeuronCores (pure data parallelism), runs a fully static
Bass/Tile program specialized on the (batch-uniform) transition schedule,
and gathers the full [B, H] output.

Device program layout (per core, b = B/8 examples):
- Everything feature-on-partition, batch on the free dim.
- tokens_h^T / tokens_c^T as [128, 2*L*b], free = h1*(L*b) + t*b + e.
- Track gates row-permuted to (i, f, o, 2g); PSUM [128, 2b]: chunk0=[i;f],
  chunk1=[o;2g].  tanh(g) computed as 2*sigmoid(2g)-1 (one sigmoid op over
  all four gates; the 2x is pre-folded into the weights).
- Tree gates (i, o, f_l, f_r, 2u) in PSUM [128, 10b], one sigmoid op.
- Biases ride augmented matmuls: th state tile is [TD+1, b] with last row 1,
  multiplied by [W_hh^T; b_ih+b_hh] and [W_x^T; b_l].
"""

import os
import sys

os.environ.setdefault("JAX_PLATFORMS", "")
if "/opt/trn_rl_repo" not in sys.path:
    sys.path.insert(0, "/opt/trn_rl_repo")

import numpy as np

N_CORES = 8
H = 256
TD = 64

# ---------------------------------------------------------------- schedule --

# Track gate rows: original (i, f, g, o); device order (i, f, o, g), g x2.
_TRACK_PERM = np.concatenate([np.arange(0, 64), np.arange(64, 128),
                              np.arange(192, 256), np.arange(128, 192)])
_TRACK_SCALE = np.concatenate([np.ones(192), np.full(64, 2.0)]).astype(np.float32)
# Tree gate rows (i, o, f_l, f_r, u); u rows x2.
_TREE_SCALE = np.concatenate([np.ones(4 * H), np.full(H, 2.0)]).astype(np.float32)


def derive_schedule(transitions: np.ndarray, L: int):
    """Symbolic stack simulation over the batch-uniform transition codes."""
    tr = np.asarray(transitions)
    if not (tr == tr[0:1]).all():
        raise NotImplementedError("non-batch-uniform transitions unsupported")
    codes = [int(c) for c in tr[0]]
    MAX_STACK = L + 2
    stack = [("tok", 0), ("tok", 0)] + [None] * (MAX_STACK - 2)
    p, bp, nred = 2, 0, 0
    steps = []
    for c in codes:
        assert 2 <= p <= MAX_STACK, f"invalid stack pointer {p}"
        top = stack[p - 1]
        sec = stack[p - 2]
        buf = ("tok", min(bp, L - 1))
        is_shift = c == 1
        is_red = c in (2, 3)
        step = dict(code=c, buf=buf, top=top, sec=sec, is_red=is_red,
                    head=None, chil=None, red_idx=None)
        if is_red:
            head, chil = (top, sec) if c == 2 else (sec, top)
            val = ("red", nred)
            step.update(head=head, chil=chil, red_idx=nred)
            nred += 1
        elif is_shift:
            val = buf
        else:
            val = top
        pos = p if is_shift else (p - 2 if is_red else p - 1)
        assert 0 <= pos < MAX_STACK
        stack[pos] = val
        p = p + int(is_shift) - int(is_red)
        bp = bp + int(is_shift)
        steps.append(step)
    return steps, stack[p - 1]


# ------------------------------------------------------------ host packing --

def _chunk_k(wt: np.ndarray) -> np.ndarray:
    """[K, M] -> [128, (K//128)*M], K-chunks stacked along the free dim."""
    K = wt.shape[0]
    assert K % 128 == 0
    return np.hstack([wt[k * 128:(k + 1) * 128] for k in range(K // 128)])


def prep_weights(W_x, U_r, U_l, b_l, W_ih, W_hh, b_ih, b_hh):
    W_ih = np.asarray(W_ih, np.float32)
    W_hh = np.asarray(W_hh, np.float32)
    sc = _TRACK_SCALE[:, None]
    W_A = W_ih[:, 0:H][_TRACK_PERM] * sc
    W_B = W_ih[:, H:2 * H][_TRACK_PERM] * sc
    W_C = W_ih[:, 2 * H:3 * H][_TRACK_PERM] * sc
    W_hh_p = W_hh[_TRACK_PERM] * sc
    btot = ((np.asarray(b_ih) + np.asarray(b_hh))[_TRACK_PERM] * _TRACK_SCALE)

    tsc = _TREE_SCALE[:, None]
    out = dict(
        wa=_chunk_k(np.ascontiguousarray(W_A.T)),                    # [128, 512]
        wb=_chunk_k(np.ascontiguousarray(W_B.T)),
        wc=_chunk_k(np.ascontiguousarray(W_C.T)),
        whh=np.vstack([W_hh_p.T, btot[None, :]]),                    # [65, 256]
        ul=_chunk_k(np.ascontiguousarray((np.asarray(U_l, np.float32) * tsc).T)),
        ur=_chunk_k(np.ascontiguousarray((np.asarray(U_r, np.float32) * tsc).T)),
        wx=np.vstack([(np.asarray(W_x, np.float32) * tsc).T,
                      (np.asarray(b_l) * _TREE_SCALE)[None, :]]),    # [65, 1280]
        ident=np.eye(128, dtype=np.float32),
    )
    return {k: np.ascontiguousarray(v, dtype=np.float32) for k, v in out.items()}


def prep_tokens(tokens: np.ndarray) -> np.ndarray:
    """[b, L, H] -> [128, 2*L*b], free = h1*(L*b) + t*b + e."""
    b, L, Hn = tokens.shape
    assert Hn == H
    arr = np.asarray(tokens, np.float32).transpose(2, 1, 0).reshape(H, L * b)
    return np.ascontiguousarray(np.hstack([arr[:128], arr[128:]]))


# ---------------------------------------------------------- device program --

def _build_program(steps, out_sym, b, L, debug_taps=False):
    import concourse.bacc as bacc
    import concourse.mybir as mybir
    import concourse.tile as tile

    f32 = mybir.dt.float32
    AF = mybir.ActivationFunctionType
    nc = bacc.Bacc("TRN2", target_bir_lowering=False, debug=False)
    Lb = L * b

    d = {}
    for name, shape in [
        ("tokh", [128, 2 * Lb]), ("tokc", [128, 2 * Lb]),
        ("wa", [128, 512]), ("wb", [128, 512]), ("wc", [128, 512]),
        ("whh", [TD + 1, 256]),
        ("ul", [128, 2 * 1280]), ("ur", [128, 2 * 1280]),
        ("wx", [TD + 1, 1280]),
        ("th0", [TD, b]), ("tc0", [TD, b]), ("ident", [128, 128]),
    ]:
        d[name] = nc.declare_dram_parameter(name, shape, f32, isOutput=False)
    d_out = nc.declare_dram_parameter("out", [b, H], f32, isOutput=True)
    d_dbg = {}
    if debug_taps:
        for name, shape in [("dbg_th", [TD, b]), ("dbg_tc", [TD, b]),
                            ("dbg_rh", [128, 2 * b]), ("dbg_rc", [128, 2 * b]),
                            ("dbg_sig", [TD, 4 * b]), ("dbg_sg", [128, 10 * b]),
                            ("dbg_psg", [128, 10 * b])]:
            d_dbg[name] = nc.declare_dram_parameter(name, shape, f32, isOutput=True)

    # Sanity-check red live ranges fit the rotating pool (bufs=3).
    red_last_use = {}
    red_birth = {}
    nred = 0
    for st in steps:
        for sym in (st["buf"], st["top"], st["sec"]):
            if sym[0] == "red":
                red_last_use[sym[1]] = max(red_last_use.get(sym[1], 0), nred)
        if st["is_red"]:
            red_birth[st["red_idx"]] = nred
            nred += 1
    if out_sym[0] == "red" and out_sym[1] in red_birth:
        red_last_use[out_sym[1]] = nred
    for r, last in red_last_use.items():
        if last - red_birth[r] > 2:
            raise NotImplementedError("red value live range too long for pool")

    with tile.TileContext(nc) as tc:
        with (
            tc.tile_pool(name="const", bufs=1) as cp,
            tc.tile_pool(name="wk", bufs=3) as wp,
            tc.tile_pool(name="pstr", bufs=2, space="PSUM") as pstr,
            tc.tile_pool(name="psgt", bufs=2, space="PSUM") as psgt,
        ):
            sb = {}
            for name in ("tokh", "tokc", "wa", "wb", "wc", "whh", "ul", "ur",
                         "wx", "ident"):
                sb[name] = cp.tile(list(d[name].shape), f32, name=f"sb_{name}")
                nc.sync.dma_start(sb[name][:], d[name].ap())

            # ping-pong state tiles; th has an extra all-ones row (bias mule)
            th_t = [cp.tile([TD + 1, b], f32, name=f"th{i}") for i in range(2)]
            tc_t = [cp.tile([TD, b], f32, name=f"tcs{i}") for i in range(2)]
            for i in range(2):
                nc.vector.memset(th_t[i][TD:TD + 1, :], 1.0)
            nc.sync.dma_start(th_t[0][0:TD, :], d["th0"].ap())
            nc.sync.dma_start(tc_t[0][:], d["tc0"].ap())

            tokh, tokc = sb["tokh"], sb["tokc"]
            red_h, red_c = {}, {}
            last_sig = last_sg = None

            def h_rhs(sym, k):
                kind, idx = sym
                if kind == "tok":
                    return tokh[:, k * Lb + idx * b: k * Lb + (idx + 1) * b]
                return red_h[idx][:, k * b:(k + 1) * b]

            def c_view(sym):
                kind, idx = sym
                if kind == "tok":
                    v = tokc[:].rearrange("p (k l b) -> p k l b", k=2, b=b)
                    return v[:, :, idx, :]
                return red_c[idx][:].rearrange("p (k b) -> p k b", k=2)

            for t, st in enumerate(steps):
                cur, nxt = t % 2, (t + 1) % 2
                # ---- tracking LSTM ----
                # PSUM [64, 4b]: gate m (i|f|o|2g) at free cols [m*b,(m+1)*b),
                # all at base partition 0 (walrus requires equal base
                # partitions for 2-SBUF-input vector ops downstream).
                ps = pstr.tile([TD, 4 * b], f32, name=f"pstr_{t}", tag="pstr")
                for m in range(4):
                    om = ps[:, m * b:(m + 1) * b]
                    first = True
                    for w_t, sym in ((sb["wa"], st["buf"]),
                                     (sb["wc"], st["sec"]),
                                     (sb["wb"], st["top"])):
                        for k in range(2):
                            nc.tensor.matmul(
                                om, w_t[:, k * 256 + m * 64: k * 256 + (m + 1) * 64],
                                h_rhs(sym, k), start=first, stop=False)
                            first = False
                    nc.tensor.matmul(om, sb["whh"][:, m * 64:(m + 1) * 64],
                                     th_t[cur][:], start=False, stop=True)
                sig = wp.tile([TD, 4 * b], f32, tag="sig", name=f"sig_{t}")
                nc.scalar.activation(sig[:], ps[:], AF.Sigmoid)
                last_sig = sig
                si, sf = sig[:, 0:b], sig[:, b:2 * b]
                so, s2g = sig[:, 2 * b:3 * b], sig[:, 3 * b:4 * b]
                At = wp.tile([TD, b], f32, tag="At", name=f"At_{t}")
                jk = wp.tile([TD, 1], f32, tag="jk", name=f"jk_{t}")
                nc.vector.affine_mul_reduce(At[:], jk[:], s2g, si, 2.0, -1.0)
                Bt = wp.tile([TD, b], f32, tag="Bt", name=f"Bt_{t}")
                nc.vector.tensor_mul(Bt[:], sf, tc_t[cur][:])
                nc.vector.tensor_add(tc_t[nxt][:], At[:], Bt[:])
                tt = wp.tile([TD, b], f32, tag="tt", name=f"tt_{t}")
                nc.scalar.activation(tt[:], tc_t[nxt][:], AF.Tanh)
                nc.vector.tensor_mul(th_t[nxt][0:TD, :], tt[:], so)

                # ---- TreeLSTM (reduce steps) ----
                if st["is_red"]:
                    psg = psgt.tile([128, 10 * b], f32, tag="psg",
                                    name=f"psg_{t}")
                    # All U matmuls (depend only on acc/tokens) before any
                    # W_x matmul (depends on th of this step): the PE is
                    # in-order, so a W_x early in the stream would stall the
                    # independent U work behind it.
                    #
                    # start=True clears has_written for the WHOLE PSUM bank,
                    # so emit it only on the first matmul touching each bank
                    # (windows 0-7 share bank0, 8-9 bank1); later windows
                    # rely on hw=0 -> overwrite semantics.
                    bank_started = set()
                    for m in range(10):
                        om = psg[:, m * b:(m + 1) * b]
                        bank = (m * b * 4) // 2048
                        for w_t, sym in ((sb["ul"], st["chil"]),
                                         (sb["ur"], st["head"])):
                            for k in range(2):
                                nc.tensor.matmul(
                                    om,
                                    w_t[:, k * 1280 + m * 128: k * 1280 + (m + 1) * 128],
                                    h_rhs(sym, k),
                                    start=bank not in bank_started, stop=False,
                                    skip_group_check=True)
                                bank_started.add(bank)
                    for m in range(10):
                        nc.tensor.matmul(psg[:, m * b:(m + 1) * b],
                                         sb["wx"][:, m * 128:(m + 1) * 128],
                                         th_t[nxt][:], start=False, stop=True,
                                         skip_group_check=True)
                    sg = wp.tile([128, 10 * b], f32, tag="sg", name=f"sg_{t}")
                    nc.scalar.activation(sg[:], psg[:], AF.Sigmoid)
                    last_sg = sg
                    if debug_taps:
                        psg_cp = cp.tile([128, 10 * b], f32, name=f"psgcp_{t}")
                        nc.scalar.copy(psg_cp[:], psg[:])
                        last_psg_cp = psg_cp
                    sgi, sgo = sg[:, 0:2 * b], sg[:, 2 * b:4 * b]
                    sfl, sfr = sg[:, 4 * b:6 * b], sg[:, 6 * b:8 * b]
                    s2u = sg[:, 8 * b:10 * b]
                    r3 = lambda ap: ap.rearrange("p (k b) -> p k b", k=2)
                    A2 = wp.tile([128, 2 * b], f32, tag="A2", name=f"A2_{t}")
                    jk2 = wp.tile([128, 1], f32, tag="jk2", name=f"jk2_{t}")
                    nc.vector.affine_mul_reduce(A2[:], jk2[:], s2u, sgi, 2.0, -1.0)
                    B2 = wp.tile([128, 2 * b], f32, tag="B2", name=f"B2_{t}")
                    nc.vector.tensor_mul(r3(B2[:]), r3(sfl), c_view(st["chil"]))
                    C2 = wp.tile([128, 2 * b], f32, tag="C2", name=f"C2_{t}")
                    nc.vector.tensor_mul(r3(C2[:]), r3(sfr), c_view(st["head"]))
                    S2 = wp.tile([128, 2 * b], f32, tag="S2", name=f"S2_{t}")
                    nc.vector.tensor_add(S2[:], A2[:], B2[:])
                    rc = wp.tile([128, 2 * b], f32, tag="rc", name=f"rc_{t}")
                    nc.vector.tensor_add(rc[:], S2[:], C2[:])
                    tt2 = wp.tile([128, 2 * b], f32, tag="tt2", name=f"tt2_{t}")
                    nc.scalar.activation(tt2[:], rc[:], AF.Tanh)
                    rh = wp.tile([128, 2 * b], f32, tag="rh", name=f"rh_{t}")
                    nc.vector.tensor_mul(rh[:], tt2[:], sgo)
                    red_h[st["red_idx"]] = rh
                    red_c[st["red_idx"]] = rc

            # ---- output: transpose [H, b] -> [b, H] and store ----
            if debug_taps:
                nt = len(steps)
                nc.sync.dma_start(d_dbg["dbg_th"].ap(),
                                  th_t[nt % 2][0:TD, :])
                nc.sync.dma_start(d_dbg["dbg_tc"].ap(), tc_t[nt % 2][:])
                if last_sig is not None:
                    nc.sync.dma_start(d_dbg["dbg_sig"].ap(), last_sig[:])
                if red_h:
                    rlast = max(red_h)
                    nc.sync.dma_start(d_dbg["dbg_rh"].ap(), red_h[rlast][:])
                    nc.sync.dma_start(d_dbg["dbg_rc"].ap(), red_c[rlast][:])
                    nc.sync.dma_start(d_dbg["dbg_sg"].ap(), last_sg[:])
                    nc.sync.dma_start(d_dbg["dbg_psg"].ap(), last_psg_cp[:])
            if out_sym[0] == "red" and out_sym[1] not in red_h:
                out_sym = ("tok", 0)  # truncated debug schedule: dummy out
            out_sb = wp.tile([b, H], f32, tag="out", name="out_sb")
            for k in range(2):
                pot = pstr.tile([b, 128], f32, tag="pout", name=f"pout_{k}")
                nc.tensor.transpose(pot[:], h_rhs(out_sym, k), sb["ident"][:])
                nc.scalar.copy(out_sb[:, k * 128:(k + 1) * 128], pot[:])
            nc.sync.dma_start(d_out.ap(), out_sb[:])

    nc.compile()
    return nc


_PROGRAM_CACHE = {}


def _get_program(codes_key, b, L, steps, out_sym):
    key = (codes_key, b, L)
    if key not in _PROGRAM_CACHE:
        _PROGRAM_CACHE[key] = _build_program(steps, out_sym, b, L)
    return _PROGRAM_CACHE[key]


# ------------------------------------------------------------------ kernel --

def kernel(**inputs) -> np.ndarray:
    from concourse.bass_utils import run_bass_kernel_spmd

    tokens_h = np.asarray(inputs["tokens_h"], np.float32)
    tokens_c = np.asarray(inputs["tokens_c"], np.float32)
    transitions = np.asarray(inputs["transitions"])
    th0 = np.asarray(inputs["th0"], np.float32)
    tc0 = np.asarray(inputs["tc0"], np.float32)
    B, L, Hn = tokens_h.shape
    assert Hn == H and B % N_CORES == 0
    b = B // N_CORES

    steps, out_sym = derive_schedule(transitions, L)
    codes_key = tuple(int(c) for c in transitions[0])
    nc = _get_program(codes_key, b, L, steps, out_sym)

    w = prep_weights(inputs["W_x"], inputs["U_r"], inputs["U_l"], inputs["b_l"],
                     inputs["W_ih"], inputs["W_hh"], inputs["b_ih"], inputs["b_hh"])
    in_maps = []
    for core in range(N_CORES):
        sl = slice(core * b, (core + 1) * b)
        m = dict(w)
        m["tokh"] = prep_tokens(tokens_h[sl])
        m["tokc"] = prep_tokens(tokens_c[sl])
        m["th0"] = np.ascontiguousarray(th0[sl].T)
        m["tc0"] = np.ascontiguousarray(tc0[sl].T)
        in_maps.append(m)

    trace = bool(int(os.environ.get("KERNEL_TRACE", "0")))
    res = run_bass_kernel_spmd(nc, in_maps, list(range(N_CORES)), trace=trace)
    if trace:
        kernel.last_exec_time_ns = res.exec_time_ns
        kernel.last_results = res
    out = np.concatenate([res.results[i]["out"] for i in range(N_CORES)], axis=0)
    return np.ascontiguousarray(out, dtype=np.float32)


# revision 16
# speedup vs baseline: 2.1820x; 2.1820x over previous
"""Trainium2 Bass kernel for nn_DependencyEncoder (stack TreeLSTM).

Self-contained: takes FULL inputs as in reference.setup_inputs(), shards the
batch across 8 NeuronCores (pure data parallelism), runs a fully static
Bass/Tile program specialized on the (batch-uniform) transition schedule,
and gathers the full [B, H] output.

Device program layout (per core, b = B/8 examples):
- Everything feature-on-partition, batch on the free dim.
- tokens_h^T / tokens_c^T as [128, 2*L*b], free = h1*(L*b) + t*b + e.
- Track gates row-permuted to (i, f, o, 2g); PSUM [128, 2b]: chunk0=[i;f],
  chunk1=[o;2g].  tanh(g) computed as 2*sigmoid(2g)-1 (one sigmoid op over
  all four gates; the 2x is pre-folded into the weights).
- Tree gates (i, o, f_l, f_r, 2u) in PSUM [128, 10b], one sigmoid op.
- Biases ride augmented matmuls: th state tile is [TD+1, b] with last row 1,
  multiplied by [W_hh^T; b_ih+b_hh] and [W_x^T; b_l].
"""

import os
import sys

os.environ.setdefault("JAX_PLATFORMS", "")
if "/opt/trn_rl_repo" not in sys.path:
    sys.path.insert(0, "/opt/trn_rl_repo")

import numpy as np
import ml_dtypes

BF16 = ml_dtypes.bfloat16
N_CORES = 8
H = 256
TD = 64

# ---------------------------------------------------------------- schedule --

# Track gate rows: original (i, f, g, o); device order (i, f, o, g), g x2.
_TRACK_PERM = np.concatenate([np.arange(0, 64), np.arange(64, 128),
                              np.arange(192, 256), np.arange(128, 192)])
_TRACK_SCALE = np.concatenate([np.ones(192), np.full(64, 2.0)]).astype(np.float32)
# Tree gate rows (i, o, f_l, f_r, u); u rows x2.
_TREE_SCALE = np.concatenate([np.ones(4 * H), np.full(H, 2.0)]).astype(np.float32)


def derive_schedule(transitions: np.ndarray, L: int):
    """Symbolic stack simulation over the batch-uniform transition codes."""
    tr = np.asarray(transitions)
    if not (tr == tr[0:1]).all():
        raise NotImplementedError("non-batch-uniform transitions unsupported")
    codes = [int(c) for c in tr[0]]
    MAX_STACK = L + 2
    stack = [("tok", 0), ("tok", 0)] + [None] * (MAX_STACK - 2)
    p, bp, nred = 2, 0, 0
    steps = []
    for c in codes:
        assert 2 <= p <= MAX_STACK, f"invalid stack pointer {p}"
        top = stack[p - 1]
        sec = stack[p - 2]
        buf = ("tok", min(bp, L - 1))
        is_shift = c == 1
        is_red = c in (2, 3)
        step = dict(code=c, buf=buf, top=top, sec=sec, is_red=is_red,
                    head=None, chil=None, red_idx=None)
        if is_red:
            head, chil = (top, sec) if c == 2 else (sec, top)
            val = ("red", nred)
            step.update(head=head, chil=chil, red_idx=nred)
            nred += 1
        elif is_shift:
            val = buf
        else:
            val = top
        pos = p if is_shift else (p - 2 if is_red else p - 1)
        assert 0 <= pos < MAX_STACK
        stack[pos] = val
        p = p + int(is_shift) - int(is_red)
        bp = bp + int(is_shift)
        steps.append(step)
    return steps, stack[p - 1]


# ------------------------------------------------------------ host packing --

def _chunk_k(wt: np.ndarray) -> np.ndarray:
    """[K, M] -> [128, (K//128)*M], K-chunks stacked along the free dim."""
    K = wt.shape[0]
    assert K % 128 == 0
    return np.hstack([wt[k * 128:(k + 1) * 128] for k in range(K // 128)])


def prep_weights(W_x, U_r, U_l, b_l, W_ih, W_hh, b_ih, b_hh):
    W_ih = np.asarray(W_ih, np.float32)
    W_hh = np.asarray(W_hh, np.float32)
    sc = _TRACK_SCALE[:, None]
    W_A = W_ih[:, 0:H][_TRACK_PERM] * sc
    W_B = W_ih[:, H:2 * H][_TRACK_PERM] * sc
    W_C = W_ih[:, 2 * H:3 * H][_TRACK_PERM] * sc
    W_hh_p = W_hh[_TRACK_PERM] * sc
    btot = ((np.asarray(b_ih) + np.asarray(b_hh))[_TRACK_PERM] * _TRACK_SCALE)

    tsc = _TREE_SCALE[:, None]
    out = dict(
        wa=_chunk_k(np.ascontiguousarray(W_A.T)),                    # [128, 512]
        wb=_chunk_k(np.ascontiguousarray(W_B.T)),
        wc=_chunk_k(np.ascontiguousarray(W_C.T)),
        whh=np.vstack([W_hh_p.T, btot[None, :]]),                    # [65, 256]
        ul=_chunk_k(np.ascontiguousarray((np.asarray(U_l, np.float32) * tsc).T)),
        ur=_chunk_k(np.ascontiguousarray((np.asarray(U_r, np.float32) * tsc).T)),
        wx=np.vstack([(np.asarray(W_x, np.float32) * tsc).T,
                      (np.asarray(b_l) * _TREE_SCALE)[None, :]]),    # [65, 1280]
        ident=np.eye(128, dtype=np.float32),
    )
    return {k: np.ascontiguousarray(v, dtype=BF16) for k, v in out.items()}


def prep_tokens(tokens: np.ndarray, dtype=np.float32) -> np.ndarray:
    """[b, L, H] -> [128, 2*L*b], free = h1*(L*b) + t*b + e."""
    b, L, Hn = tokens.shape
    assert Hn == H
    arr = np.asarray(tokens, np.float32).transpose(2, 1, 0).reshape(H, L * b)
    return np.ascontiguousarray(np.hstack([arr[:128], arr[128:]]).astype(dtype))


# ---------------------------------------------------------- device program --

def _build_program(steps, out_sym, b, L, debug_taps=False):
    import concourse.bacc as bacc
    import concourse.mybir as mybir
    import concourse.tile as tile

    f32 = mybir.dt.float32
    bf16 = mybir.dt.bfloat16
    AF = mybir.ActivationFunctionType
    nc = bacc.Bacc("TRN2", target_bir_lowering=False, debug=False)
    Lb = L * b

    d = {}
    DTYPES = {"tokc": f32, "tc0": f32}
    for name, shape in [
        ("tokh", [128, 2 * Lb]), ("tokc", [128, 2 * Lb]),
        ("wa", [128, 512]), ("wb", [128, 512]), ("wc", [128, 512]),
        ("whh", [TD + 1, 256]),
        ("ul", [128, 2 * 1280]), ("ur", [128, 2 * 1280]),
        ("wx", [TD + 1, 1280]),
        ("th0", [TD, b]), ("tc0", [TD, b]), ("ident", [128, 128]),
    ]:
        d[name] = nc.declare_dram_parameter(name, shape, DTYPES.get(name, bf16),
                                            isOutput=False)
    d_out = nc.declare_dram_parameter("out", [b, H], f32, isOutput=True)
    d_dbg = {}
    if debug_taps:
        for name, shape in [("dbg_th", [TD, b]), ("dbg_tc", [TD, b]),
                            ("dbg_rh", [128, 2 * b]), ("dbg_rc", [128, 2 * b]),
                            ("dbg_sig", [TD, 4 * b]), ("dbg_sg", [128, 10 * b]),
                            ("dbg_psg", [128, 10 * b])]:
            d_dbg[name] = nc.declare_dram_parameter(name, shape, f32, isOutput=True)

    # Sanity-check red live ranges fit the rotating pool (bufs=3).
    red_last_use = {}
    red_birth = {}
    nred = 0
    for st in steps:
        for sym in (st["buf"], st["top"], st["sec"]):
            if sym[0] == "red":
                red_last_use[sym[1]] = max(red_last_use.get(sym[1], 0), nred)
        if st["is_red"]:
            red_birth[st["red_idx"]] = nred
            nred += 1
    if out_sym[0] == "red" and out_sym[1] in red_birth:
        red_last_use[out_sym[1]] = nred
    for r, last in red_last_use.items():
        if last - red_birth[r] > 2:
            raise NotImplementedError("red value live range too long for pool")

    with tile.TileContext(nc) as tc:
        with (
            tc.tile_pool(name="const", bufs=1) as cp,
            tc.tile_pool(name="wk", bufs=3) as wp,
            tc.tile_pool(name="pstr", bufs=2, space="PSUM") as pstr,
            tc.tile_pool(name="psgt", bufs=2, space="PSUM") as psgt,
        ):
            sb = {}
            for name in ("tokh", "tokc", "wa", "wb", "wc", "whh", "ul", "ur",
                         "wx", "ident"):
                sb[name] = cp.tile(list(d[name].shape), DTYPES.get(name, bf16),
                                   name=f"sb_{name}")
                nc.sync.dma_start(sb[name][:], d[name].ap())

            # ping-pong state tiles; th has an extra all-ones row (bias mule)
            th_t = [cp.tile([TD + 1, b], bf16, name=f"th{i}") for i in range(2)]
            tc_t = [cp.tile([TD, b], f32, name=f"tcs{i}") for i in range(2)]
            for i in range(2):
                nc.vector.memset(th_t[i][TD:TD + 1, :], 1.0)
            nc.sync.dma_start(th_t[0][0:TD, :], d["th0"].ap())
            nc.sync.dma_start(tc_t[0][:], d["tc0"].ap())

            tokh, tokc = sb["tokh"], sb["tokc"]
            red_h, red_c = {}, {}
            last_sig = last_sg = None

            def h_rhs(sym, k):
                kind, idx = sym
                if kind == "tok":
                    return tokh[:, k * Lb + idx * b: k * Lb + (idx + 1) * b]
                return red_h[idx][:, k * b:(k + 1) * b]

            def c_view(sym):
                kind, idx = sym
                if kind == "tok":
                    v = tokc[:].rearrange("p (k l b) -> p k l b", k=2, b=b)
                    return v[:, :, idx, :]
                return red_c[idx][:].rearrange("p (k b) -> p k b", k=2)

            for t, st in enumerate(steps):
                cur, nxt = t % 2, (t + 1) % 2
                # ---- tracking LSTM ----
                # PSUM [64, 4b]: gate m (i|f|o|2g) at free cols [m*b,(m+1)*b),
                # all at base partition 0 (walrus requires equal base
                # partitions for 2-SBUF-input vector ops downstream).
                ps = pstr.tile([TD, 4 * b], f32, name=f"pstr_{t}", tag="pstr")
                for m in range(4):
                    om = ps[:, m * b:(m + 1) * b]
                    first = True
                    for w_t, sym in ((sb["wa"], st["buf"]),
                                     (sb["wc"], st["sec"]),
                                     (sb["wb"], st["top"])):
                        for k in range(2):
                            nc.tensor.matmul(
                                om, w_t[:, k * 256 + m * 64: k * 256 + (m + 1) * 64],
                                h_rhs(sym, k), start=first, stop=False)
                            first = False
                    nc.tensor.matmul(om, sb["whh"][:, m * 64:(m + 1) * 64],
                                     th_t[cur][:], start=False, stop=True)
                sig = wp.tile([TD, 4 * b], f32, tag="sig", name=f"sig_{t}")
                nc.scalar.activation(sig[:], ps[:], AF.Sigmoid)
                last_sig = sig
                si, sf = sig[:, 0:b], sig[:, b:2 * b]
                so, s2g = sig[:, 2 * b:3 * b], sig[:, 3 * b:4 * b]
                At = wp.tile([TD, b], f32, tag="At", name=f"At_{t}")
                jk = wp.tile([TD, 1], f32, tag="jk", name=f"jk_{t}")
                nc.vector.affine_mul_reduce(At[:], jk[:], s2g, si, 2.0, -1.0)
                Bt = wp.tile([TD, b], f32, tag="Bt", name=f"Bt_{t}")
                nc.vector.tensor_mul(Bt[:], sf, tc_t[cur][:])
                nc.vector.tensor_add(tc_t[nxt][:], At[:], Bt[:])
                tt = wp.tile([TD, b], f32, tag="tt", name=f"tt_{t}")
                nc.scalar.activation(tt[:], tc_t[nxt][:], AF.Tanh)
                nc.vector.tensor_mul(th_t[nxt][0:TD, :], tt[:], so)

                # ---- TreeLSTM (reduce steps) ----
                if st["is_red"]:
                    psg = psgt.tile([128, 10 * b], f32, tag="psg",
                                    name=f"psg_{t}")
                    # All U matmuls (depend only on acc/tokens) before any
                    # W_x matmul (depends on th of this step): the PE is
                    # in-order, so a W_x early in the stream would stall the
                    # independent U work behind it.
                    #
                    # start=True clears has_written for the WHOLE PSUM bank,
                    # so emit it only on the first matmul touching each bank
                    # (windows 0-7 share bank0, 8-9 bank1); later windows
                    # rely on hw=0 -> overwrite semantics.
                    bank_started = set()
                    for m in range(10):
                        om = psg[:, m * b:(m + 1) * b]
                        bank = (m * b * 4) // 2048
                        for w_t, sym in ((sb["ul"], st["chil"]),
                                         (sb["ur"], st["head"])):
                            for k in range(2):
                                nc.tensor.matmul(
                                    om,
                                    w_t[:, k * 1280 + m * 128: k * 1280 + (m + 1) * 128],
                                    h_rhs(sym, k),
                                    start=bank not in bank_started, stop=False,
                                    skip_group_check=True)
                                bank_started.add(bank)
                    for m in range(10):
                        nc.tensor.matmul(psg[:, m * b:(m + 1) * b],
                                         sb["wx"][:, m * 128:(m + 1) * 128],
                                         th_t[nxt][:], start=False, stop=True,
                                         skip_group_check=True)
                    sg = wp.tile([128, 10 * b], f32, tag="sg", name=f"sg_{t}")
                    nc.scalar.activation(sg[:], psg[:], AF.Sigmoid)
                    last_sg = sg
                    if debug_taps:
                        psg_cp = cp.tile([128, 10 * b], f32, name=f"psgcp_{t}")
                        nc.scalar.copy(psg_cp[:], psg[:])
                        last_psg_cp = psg_cp
                    sgi, sgo = sg[:, 0:2 * b], sg[:, 2 * b:4 * b]
                    sfl, sfr = sg[:, 4 * b:6 * b], sg[:, 6 * b:8 * b]
                    s2u = sg[:, 8 * b:10 * b]
                    r3 = lambda ap: ap.rearrange("p (k b) -> p k b", k=2)
                    A2 = wp.tile([128, 2 * b], f32, tag="A2", name=f"A2_{t}")
                    jk2 = wp.tile([128, 1], f32, tag="jk2", name=f"jk2_{t}")
                    nc.vector.affine_mul_reduce(A2[:], jk2[:], s2u, sgi, 2.0, -1.0)
                    B2 = wp.tile([128, 2 * b], f32, tag="B2", name=f"B2_{t}")
                    nc.vector.tensor_mul(r3(B2[:]), r3(sfl), c_view(st["chil"]))
                    C2 = wp.tile([128, 2 * b], f32, tag="C2", name=f"C2_{t}")
                    nc.vector.tensor_mul(r3(C2[:]), r3(sfr), c_view(st["head"]))
                    S2 = wp.tile([128, 2 * b], f32, tag="S2", name=f"S2_{t}")
                    nc.vector.tensor_add(S2[:], A2[:], B2[:])
                    rc = wp.tile([128, 2 * b], f32, tag="rc", name=f"rc_{t}")
                    nc.vector.tensor_add(rc[:], S2[:], C2[:])
                    tt2 = wp.tile([128, 2 * b], f32, tag="tt2", name=f"tt2_{t}")
                    nc.scalar.activation(tt2[:], rc[:], AF.Tanh)
                    rh = wp.tile([128, 2 * b], bf16, tag="rh", name=f"rh_{t}")
                    nc.vector.tensor_mul(rh[:], tt2[:], sgo)
                    red_h[st["red_idx"]] = rh
                    red_c[st["red_idx"]] = rc

            # ---- output: transpose [H, b] -> [b, H] and store ----
            if debug_taps:
                nt = len(steps)
                nc.sync.dma_start(d_dbg["dbg_th"].ap(),
                                  th_t[nt % 2][0:TD, :])
                nc.sync.dma_start(d_dbg["dbg_tc"].ap(), tc_t[nt % 2][:])
                if last_sig is not None:
                    nc.sync.dma_start(d_dbg["dbg_sig"].ap(), last_sig[:])
                if red_h:
                    rlast = max(red_h)
                    nc.sync.dma_start(d_dbg["dbg_rh"].ap(), red_h[rlast][:])
                    nc.sync.dma_start(d_dbg["dbg_rc"].ap(), red_c[rlast][:])
                    nc.sync.dma_start(d_dbg["dbg_sg"].ap(), last_sg[:])
                    nc.sync.dma_start(d_dbg["dbg_psg"].ap(), last_psg_cp[:])
            if out_sym[0] == "red" and out_sym[1] not in red_h:
                out_sym = ("tok", 0)  # truncated debug schedule: dummy out
            out_sb = wp.tile([b, H], f32, tag="out", name="out_sb")
            out_dt = bf16 if out_sym[0] == "red" else bf16
            for k in range(2):
                pot = pstr.tile([b, 128], out_dt, tag="pout", name=f"pout_{k}")
                nc.tensor.transpose(pot[:], h_rhs(out_sym, k), sb["ident"][:])
                nc.scalar.copy(out_sb[:, k * 128:(k + 1) * 128], pot[:])
            nc.sync.dma_start(d_out.ap(), out_sb[:])

    nc.compile()
    return nc


_PROGRAM_CACHE = {}


def _get_program(codes_key, b, L, steps, out_sym):
    key = (codes_key, b, L)
    if key not in _PROGRAM_CACHE:
        _PROGRAM_CACHE[key] = _build_program(steps, out_sym, b, L)
    return _PROGRAM_CACHE[key]


# ------------------------------------------------------------------ kernel --

def kernel(**inputs) -> np.ndarray:
    from concourse.bass_utils import run_bass_kernel_spmd

    tokens_h = np.asarray(inputs["tokens_h"], np.float32)
    tokens_c = np.asarray(inputs["tokens_c"], np.float32)
    transitions = np.asarray(inputs["transitions"])
    th0 = np.asarray(inputs["th0"], np.float32)
    tc0 = np.asarray(inputs["tc0"], np.float32)
    B, L, Hn = tokens_h.shape
    assert Hn == H and B % N_CORES == 0
    b = B // N_CORES

    steps, out_sym = derive_schedule(transitions, L)
    codes_key = tuple(int(c) for c in transitions[0])
    nc = _get_program(codes_key, b, L, steps, out_sym)

    w = prep_weights(inputs["W_x"], inputs["U_r"], inputs["U_l"], inputs["b_l"],
                     inputs["W_ih"], inputs["W_hh"], inputs["b_ih"], inputs["b_hh"])
    in_maps = []
    for core in range(N_CORES):
        sl = slice(core * b, (core + 1) * b)
        m = dict(w)
        m["tokh"] = prep_tokens(tokens_h[sl], BF16)
        m["tokc"] = prep_tokens(tokens_c[sl])
        m["th0"] = np.ascontiguousarray(th0[sl].T.astype(BF16))
        m["tc0"] = np.ascontiguousarray(tc0[sl].T)
        in_maps.append(m)

    trace = bool(int(os.environ.get("KERNEL_TRACE", "0")))
    res = run_bass_kernel_spmd(nc, in_maps, list(range(N_CORES)), trace=trace)
    if trace:
        kernel.last_exec_time_ns = res.exec_time_ns
        kernel.last_results = res
    out = np.concatenate([res.results[i]["out"] for i in range(N_CORES)], axis=0)
    return np.ascontiguousarray(out, dtype=np.float32)
